# revision 1
# baseline (speedup 1.0000x reference)
"""Single-head self-attention (B=4, S=2048, D=1024) on 8 Trainium2 NeuronCores.

v3: pair tensor-parallel K/V. Core c handles batch b = c//2 and query half
h = c%2. Unlike v2 (which recomputed full-sequence K/V on both cores of a
pair), each core projects K/V only for its OWN sequence half and the pair
exchanges halves with AllGather collectives (replica groups
[[0,1],[2,3],[4,5],[6,7]]), cutting 256 of the 1184 512-row matmuls.

Exchange schedule (measured ~15us launch + ~13us/MB per collective, all
pair groups concurrent, collectives serialize with each other):
  K-half phase (128 mm) -> ccK (2MB in -> 4MB out)
  V-half eb=0  ( 64 mm) -> ccV1 (1MB -> 2MB)
  V-half eb=1  ( 64 mm) -> ccV2 (1MB -> 2MB)
  QT phase     (128 mm)
  attention    (552 mm)  needs: full KT at ~t+96us (ccK done ~76),
                         V[:, 0:512] at ~124 (ccV1 ~104), V[:, 512:] at ~138
                         (ccV2 ~132).
KT/V are reloaded from the gathered DRAM buffers in GLOBAL key order (the
same for both pair members, so the SPMD program stays uniform; softmax is
permutation-invariant over keys so key order just has to match between KT
and V). All collective-dependent loads ride the sync ring AFTER the bounce
stores (a waiting DMA blocks its in-order engine queue, and sync has
nothing left to do); O stores + the WK/WV/WQ input loads ride the ScalarE
ring so startup is fed by both rings. All eb=1 O chains run after both
groups' eb=0 chains, giving the ccV2-fed V[:, 512:] ~30us of slack.

Everything else is bf16 storage with fp32 PSUM/rowsum/output: rowsum
matmuls lag TWO ST chains (engine-to-engine semaphore hops ~0.5us + exp
~0.7us make an exp usable only ~2us after its chain), stps/opsp hold 3
PSUM bufs, and each group's rowsum transpose + reciprocal hide under its
first O chains. One unified projection PSUM pool (single tag) avoids
pool-transition barriers between the K/V/QT phases, and the kt loads are
split across both rings (their ccK wait clears before the ScalarE queue's
exps are needed; V loads must stay sync-only). Verified on HW: 261.4us
clean-clock, rel_err 3.4e-3 (gate 2e-2).
"""

import numpy as np
from contextlib import ExitStack

import ml_dtypes

import concourse.tile as tile
from concourse import bacc, mybir
from concourse.bass_utils import run_bass_kernel_spmd

F32 = mybir.dt.float32
BF16 = mybir.dt.bfloat16
EXP = mybir.ActivationFunctionType.Exp

B, S, D = 4, 2048, 1024
NQ = 1024          # query rows / sequence-half per core
QG = 512           # q-group width for the attention passes
NGROUPS = NQ // QG
NET = D // 128     # 8 e-tiles
NDT = D // 128     # 8 d-tiles (contraction)
NKT = S // 128     # 16 k-tiles (full key sequence)
SCALE = 1.0 / float(np.sqrt(D))
GROUPS = [[0, 1], [2, 3], [4, 5], [6, 7]]

_CACHE = {}


def _build_nc():
    nc = bacc.Bacc("TRN2", target_bir_lowering=False, debug=False)

    xt_d = nc.dram_tensor("xt", [D, NQ], BF16, kind="ExternalInput")
    wq_d = nc.dram_tensor("wq", [D, D], BF16, kind="ExternalInput")
    wk_d = nc.dram_tensor("wk", [D, D], BF16, kind="ExternalInput")
    wv_d = nc.dram_tensor("wv", [D, D], BF16, kind="ExternalInput")
    ones16_d = nc.dram_tensor("ones16", [128, 2], BF16, kind="ExternalInput")
    ones32_d = nc.dram_tensor("ones32", [1, 2], F32, kind="ExternalInput")
    o_d = nc.dram_tensor("o", [NQ, D], F32, kind="ExternalOutput")

    def dslc(dt_):
        return slice(dt_ * 128, (dt_ + 1) * 128)

    with tile.TileContext(nc) as tc, ExitStack() as ctx:
        dram = ctx.enter_context(tc.tile_pool(name="dram", bufs=1,
                                              space="DRAM"))
        kb_d = dram.tile([D, NQ], BF16, name="kb_d", tag="kb_d")
        kout_d = dram.tile([2 * D, NQ], BF16, name="kout_d", tag="kout_d")
        vb1_d = dram.tile([NQ, 512], BF16, name="vb1_d", tag="vb1_d")
        vout1_d = dram.tile([2 * NQ, 512], BF16, name="vout1_d", tag="vout1_d")
        vb2_d = dram.tile([NQ, 512], BF16, name="vb2_d", tag="vb2_d")
        vout2_d = dram.tile([2 * NQ, 512], BF16, name="vout2_d", tag="vout2_d")

        small = ctx.enter_context(tc.tile_pool(name="small", bufs=1))
        ones16 = small.tile([128, 2], BF16, name="ones16", tag="ones16")
        nc.sync.dma_start(ones16[:], ones16_d.ap())
        ones32 = small.tile([1, 2], F32, name="ones32", tag="ones32")
        nc.sync.dma_start(ones32[:], ones32_d.ap())
        exp_warm = small.tile([1, 2], F32, name="exp_warm", tag="exp_warm")
        nc.scalar.activation(exp_warm[:], ones32[:], EXP, bias=0.0, scale=1.0)

        # Projection-phase operands (right stack, released before attention).
        xres = tc.alloc_tile_pool(name="xres", bufs=1, side="right")
        wkp = tc.alloc_tile_pool(name="wkp", bufs=1, side="right")
        wvp = tc.alloc_tile_pool(name="wvp", bufs=1, side="right")
        wqp = tc.alloc_tile_pool(name="wqp", bufs=1, side="right")
        stgp = tc.alloc_tile_pool(name="stgp", bufs=6, side="right")

        xt_sb = [xres.tile([128, NQ], BF16, name=f"xtile{dt_}",
                           tag=f"xtile{dt_}")
                 for dt_ in range(NDT)]
        wk_sb = [wkp.tile([128, D], BF16, name=f"wk{dt_}", tag=f"wk{dt_}")
                 for dt_ in range(NDT)]
        wv_sb = [wvp.tile([128, D], BF16, name=f"wv{dt_}", tag=f"wv{dt_}")
                 for dt_ in range(NDT)]
        wq_sb = [wqp.tile([128, D], BF16, name=f"wq{dt_}", tag=f"wq{dt_}")
                 for dt_ in range(NDT)]

        # Input loads split across BOTH rings so the K phase is never
        # DMA-paced at startup (a single ring ramps too slowly): weights WK/
        # WV ride the ScalarE ring (idle until the O stores much later), xT
        # and WQ ride the sync ring. All transfers keep >=1KB partition
        # lines (256B lines run the ring at ~50GB/s vs ~390GB/s).
        for half in range(2):
            cs = slice(half * 512, (half + 1) * 512)
            for dt_ in range(NDT):
                nc.scalar.dma_start(wk_sb[dt_][:, cs], wk_d.ap()[dslc(dt_), cs])
        for dt_ in range(NDT):
            nc.sync.dma_start(xt_sb[dt_][:, 0:512], xt_d.ap()[dslc(dt_), 0:512])
        for dt_ in range(NDT):
            nc.sync.dma_start(xt_sb[dt_][:, 512:1024],
                              xt_d.ap()[dslc(dt_), 512:1024])
        for eb in range(2):
            cs = slice(eb * 512, (eb + 1) * 512)
            for dt_ in range(NDT):
                nc.scalar.dma_start(wv_sb[dt_][:, cs], wv_d.ap()[dslc(dt_), cs])
        for half in range(2):
            cs = slice(half * 512, (half + 1) * 512)
            for dt_ in range(NDT):
                nc.scalar.dma_start(wq_sb[dt_][:, cs], wq_d.ap()[dslc(dt_), cs])

        # Long-lived residents.
        qres = ctx.enter_context(tc.tile_pool(name="qres", bufs=1))
        qt_sb = [qres.tile([128, NQ], BF16, name=f"qtile{et}", tag=f"qtile{et}")
                 for et in range(NET)]
        vres = ctx.enter_context(tc.tile_pool(name="vres", bufs=1))
        v_sb = [vres.tile([128, D], BF16, name=f"vtile{st}", tag=f"vtile{st}")
                for st in range(NKT)]
        kres = ctx.enter_context(tc.tile_pool(name="kres", bufs=1))
        kt_sb = [kres.tile([128, S], BF16, name=f"ktile{et}", tag=f"ktile{et}")
                 for et in range(NET)]

        warmp = tc.alloc_tile_pool(name="warmp", bufs=1, space="PSUM")
        warm_ps = warmp.tile([1, 2], F32, name="warm_ps", tag="warm_ps")
        for _ in range(64):
            nc.tensor.matmul(warm_ps[:], ones16[:, 0:1], ones16[:, 0:2],
                             start=True, stop=True)

        # ---- Phase 1: K own-half -> bounce ----
        pps = tc.alloc_tile_pool(name="pps", bufs=6, space="PSUM")
        for et in range(NET):
            for kb in range(NQ // 512):
                ps = pps.tile([128, 512], F32, name="pp", tag="pp")
                for dt_ in range(NDT):
                    nc.tensor.matmul(
                        ps[:],
                        wk_sb[dt_][:, et * 128:(et + 1) * 128],
                        xt_sb[dt_][:, kb * 512:(kb + 1) * 512],
                        start=(dt_ == 0), stop=(dt_ == NDT - 1))
                stg = stgp.tile([128, 512], BF16, name="kstg", tag="kstg")
                nc.vector.tensor_copy(stg[:], ps[:])
                nc.sync.dma_start(
                    kb_d[:][et * 128:(et + 1) * 128, kb * 512:(kb + 1) * 512],
                    stg[:])

        nc.gpsimd.collective_compute(
            "AllGather", mybir.AluOpType.bypass, replica_groups=GROUPS,
            ins=[kb_d.opt()], outs=[kout_d.opt()])

        # ---- Phase 2: V own-half (eb outer so ccV1 launches early) ----
        for eb in range(2):
            vb = vb1_d if eb == 0 else vb2_d
            for st in range(NQ // 128):
                ps = pps.tile([128, 512], F32, name="pp2", tag="pp")
                for dt_ in range(NDT):
                    nc.tensor.matmul(
                        ps[:],
                        xt_sb[dt_][:, st * 128:(st + 1) * 128],
                        wv_sb[dt_][:, eb * 512:(eb + 1) * 512],
                        start=(dt_ == 0), stop=(dt_ == NDT - 1))
                stg = stgp.tile([128, 512], BF16, name="vstg", tag="vstg")
                nc.vector.tensor_copy(stg[:], ps[:])
                nc.sync.dma_start(
                    vb[:][st * 128:(st + 1) * 128, :], stg[:])
            nc.gpsimd.collective_compute(
                "AllGather", mybir.AluOpType.bypass, replica_groups=GROUPS,
                ins=[(vb1_d if eb == 0 else vb2_d).opt()],
                outs=[(vout1_d if eb == 0 else vout2_d).opt()])

        # kt loads are split across both rings (their ccK wait clears
        # before anything later on the ScalarE queue -- the attention exps
        # -- is needed; the V loads must stay sync-only, since a ccV1/ccV2
        # wait on the ScalarE queue would block those exps). Splitting
        # halves the kt drain after ccK lands. Global key/seq order: rows
        # 0:D of a gathered buffer = pair rank 0 = global half 0, so loads
        # are uniform across cores.
        for et in range(NET):
            eng = nc.sync if et % 2 == 0 else nc.scalar
            eng.dma_start(kt_sb[et][:, 0:NQ],
                          kout_d[:][et * 128:(et + 1) * 128, :])
            eng.dma_start(kt_sb[et][:, NQ:S],
                          kout_d[:][D + et * 128:D + (et + 1) * 128, :])
        for st in range(NKT):
            nc.sync.dma_start(v_sb[st][:, 0:512],
                              vout1_d[:][st * 128:(st + 1) * 128, :])
        for st in range(NKT):
            nc.sync.dma_start(v_sb[st][:, 512:1024],
                              vout2_d[:][st * 128:(st + 1) * 128, :])

        # ---- Phase 3: QT resident ----
        for qb in range(NQ // 512):
            for et in range(NET):
                ps = pps.tile([128, 512], F32, name="pp3", tag="pp")
                for dt_ in range(NDT):
                    nc.tensor.matmul(
                        ps[:],
                        wq_sb[dt_][:, et * 128:(et + 1) * 128],
                        xt_sb[dt_][:, qb * 512:(qb + 1) * 512],
                        start=(dt_ == 0), stop=(dt_ == NDT - 1))
                nc.vector.tensor_copy(
                    qt_sb[et][:, qb * 512:(qb + 1) * 512], ps[:])

        stgp.release()
        wqp.release()
        wvp.release()
        wkp.release()
        xres.release()
        pps.release()

        warmp.release()

        # ---- Attention: per q-group (ST -> exp -> rowsum -> O) ----
        # Engine-to-engine semaphore hops cost ~0.5us, the exp ~0.7us, so
        # an exp result is usable only ~2us after its ST chain ends: rowsum
        # matmuls lag TWO chains behind, stps/opsp hold 3 bufs each.
        with tc.tile_pool(name="attp", bufs=2) as attp, \
             tc.tile_pool(name="osbp", bufs=3) as osbp, \
             tc.tile_pool(name="rssb", bufs=2) as rssb, \
             tc.tile_pool(name="stps", bufs=3, space="PSUM") as stps, \
             tc.tile_pool(name="rsps", bufs=1, space="PSUM") as rsps, \
             tc.tile_pool(name="opsp", bufs=3, space="PSUM") as opsp:

            def o_chain(g, pt_strip, qtl, eb):
                ps = opsp.tile([128, 512], F32, name="o_ps", tag="o_ps")
                for kt in range(NKT):
                    nc.tensor.matmul(
                        ps[:],
                        pt_strip[kt][:, qtl * 128:(qtl + 1) * 128],
                        v_sb[kt][:, eb * 512:(eb + 1) * 512],
                        start=(kt == 0), stop=(kt == NKT - 1))
                return ps

            def o_scale(g, ps, rs_sb, qtl, eb):
                osb = osbp.tile([128, 512], F32, name="o_sb", tag="o_sb")
                nc.vector.tensor_scalar_mul(
                    osb[:], ps[:], rs_sb[:, qtl:qtl + 1])
                nc.scalar.dma_start(
                    o_d.ap()[g * QG + qtl * 128:g * QG + (qtl + 1) * 128,
                             eb * 512:(eb + 1) * 512],
                    osb[:])

            saved = []
            for g in range(NGROUPS):
                qslc = slice(g * QG, (g + 1) * QG)

                rs_row_ps = rsps.tile([1, QG], F32, name="rs_row_ps",
                                      tag="rs_row_ps")
                pt_strip = []
                for kt in range(NKT):
                    ps = stps.tile([128, QG], F32, name="st_ps", tag="st_ps")
                    for et in range(NET):
                        nc.tensor.matmul(
                            ps[:],
                            kt_sb[et][:, kt * 128:(kt + 1) * 128],
                            qt_sb[et][:, qslc],
                            start=(et == 0), stop=(et == NET - 1))
                    pt = attp.tile([128, QG], BF16, name=f"pt{kt}",
                                   tag=f"pt{kt}")
                    nc.scalar.activation(pt[:], ps[:], EXP, bias=0.0,
                                         scale=SCALE)
                    pt_strip.append(pt)
                    if kt > 1:
                        nc.tensor.matmul(
                            rs_row_ps[:],
                            ones16[:, 0:1],
                            pt_strip[kt - 2][:],
                            start=(kt == 2), stop=False)

                # Final two rs matmuls + rowsum transpose + reciprocal hide
                # under the first eb=0 O chains. All eb=1 O chains are
                # deferred until BOTH groups' eb=0 work is done, so the
                # ccV2-fed V[:, 512:] is needed ~30us later than eb=0.
                ps00 = o_chain(g, pt_strip, 0, 0)
                nc.tensor.matmul(rs_row_ps[:], ones16[:, 0:1],
                                 pt_strip[NKT - 2][:], start=False, stop=False)
                ps10 = o_chain(g, pt_strip, 1, 0)
                nc.tensor.matmul(rs_row_ps[:], ones16[:, 0:1],
                                 pt_strip[NKT - 1][:], start=False, stop=True)

                rs_row_sb = rssb.tile([1, QG], F32, name="rs_row_sb",
                                      tag="rs_row_sb")
                nc.vector.tensor_copy(rs_row_sb[:], rs_row_ps[:])
                rs_t_ps = rsps.tile([128, 2 * (QG // 128)], F32,
                                    name="rs_t_ps", tag="rs_t_ps")
                for qtl in range(QG // 128):
                    nc.tensor.matmul(
                        rs_t_ps[:, 2 * qtl:2 * qtl + 2],
                        rs_row_sb[:, qtl * 128:(qtl + 1) * 128],
                        ones32[:],
                        start=True, stop=True)
                rs_sb = rssb.tile([128, QG // 128], F32, name="rs_sb",
                                  tag="rs_sb")
                for qtl in range(QG // 128):
                    nc.vector.reciprocal(rs_sb[:, qtl:qtl + 1],
                                         rs_t_ps[:, 2 * qtl:2 * qtl + 1])

                o_scale(g, ps00, rs_sb, 0, 0)
                o_scale(g, ps10, rs_sb, 1, 0)
                for qtl in range(2, QG // 128):
                    ps = o_chain(g, pt_strip, qtl, 0)
                    o_scale(g, ps, rs_sb, qtl, 0)
                saved.append((pt_strip, rs_sb))

            for g in range(NGROUPS):
                pt_strip, rs_sb = saved[g]
                for qtl in range(QG // 128):
                    ps = o_chain(g, pt_strip, qtl, 1)
                    o_scale(g, ps, rs_sb, qtl, 1)

    nc.compile()
    return nc


def get_nc():
    if "nc" not in _CACHE:
        _CACHE["nc"] = _build_nc()
    return _CACHE["nc"]


def make_in_maps(x, WQ, WK, WV):
    bf16 = ml_dtypes.bfloat16
    ones16 = np.ones((128, 2), bf16)
    ones32 = np.ones((1, 2), np.float32)
    wq16 = np.ascontiguousarray(np.asarray(WQ, np.float32).astype(bf16))
    wk16 = np.ascontiguousarray(np.asarray(WK, np.float32).astype(bf16))
    wv16 = np.ascontiguousarray(np.asarray(WV, np.float32).astype(bf16))
    in_maps = []
    for c in range(8):
        b, h = c // 2, c % 2
        xT = np.asarray(x[b], np.float32).T[:, h * NQ:(h + 1) * NQ].astype(bf16)
        in_maps.append({"xt": np.ascontiguousarray(xT),
                        "wq": wq16, "wk": wk16, "wv": wv16,
                        "ones16": ones16, "ones32": ones32})
    return in_maps


def kernel(**inputs):
    x = np.asarray(inputs["x"], dtype=np.float32)
    WQ = np.asarray(inputs["WQ"], dtype=np.float32)
    WK = np.asarray(inputs["WK"], dtype=np.float32)
    WV = np.asarray(inputs["WV"], dtype=np.float32)

    nc = get_nc()
    in_maps = make_in_maps(x, WQ, WK, WV)
    res = run_bass_kernel_spmd(nc, in_maps, core_ids=list(range(8)))

    out = np.empty((B, S, D), np.float32)
    for c in range(8):
        b, h = c // 2, c % 2
        out[b, h * NQ:(h + 1) * NQ, :] = res.results[c]["o"]
    return out


if __name__ == "__main__":
    rng = np.random.default_rng(0)
    x = rng.standard_normal((B, S, D), dtype=np.float32)
    WQ = (rng.standard_normal((D, D), dtype=np.float32) * 0.02)
    WK = (rng.standard_normal((D, D), dtype=np.float32) * 0.02)
    WV = (rng.standard_normal((D, D), dtype=np.float32) * 0.02)
    o = kernel(x=x, WQ=WQ, WK=WK, WV=WV)
    print("out", o.shape, o.dtype, float(np.abs(o).max()))

